# revision 5
# baseline (speedup 1.0000x reference)
"""DiagonalUpsample Trainium2 kernel.

Interleave two [B,C,H,W] f32 tensors into [B,C,2H,2W]:
  out[2i,2j]   = d[i,j]    out[2i,2j+1] = u[i,j]
  out[2i+1,2j] = u[i,j]    out[2i+1,2j+1] = d[i,j]

Flattened over rows r = (b,c,i):  out_row[2r] = interleave(d_r, u_r),
out_row[2r+1] = interleave(u_r, d_r).  Sharded batch-parallel across 8
NeuronCores (core k takes batch k).  Per core: 16 iterations; each DMAs
1 MiB of u and d into SBUF ([128 partitions x 2048 f32], partition p
holding 8 consecutive input rows), builds the interleaved output tile
[128 x 8192] with 4 strided copies (DVE + ACT), and stores it with a
single fully-contiguous 4 MiB DMA (partition p's 32 KiB lands exactly at
output rows 16p..16p+15 of the chunk).
"""

import sys

import numpy as np

try:
    import concourse.bass as bass  # noqa: F401
except ImportError:  # grading env may not have the repo on sys.path
    sys.path.insert(0, "/opt/trn_rl_repo")

from contextlib import ExitStack

import concourse.bacc as bacc
import concourse.bass as bass
import concourse.tile as tile
from concourse import mybir
from concourse import bass_utils

B, C, H, W = 8, 64, 256, 256
N_CORES = 8
P = 128                      # SBUF partitions
ROWS_PER_CORE = C * H        # 16384 input rows of W f32 per core
K = 8                        # input rows per partition per iteration
ROWS_PER_ITER = P * K        # 1024
N_ITER = ROWS_PER_CORE // ROWS_PER_ITER  # 16
FIN = K * W                  # 2048 f32 per partition in
FOUT = 4 * FIN               # 8192 f32 per partition out

_cache = {}


def _build_nc():
    nc = bacc.Bacc(
        "TRN2",
        target_bir_lowering=False,
        debug=False,
        enable_asserts=False,
    )
    f32 = mybir.dt.float32
    u = nc.dram_tensor("u", [ROWS_PER_CORE * W], f32, kind="ExternalInput").ap()
    d = nc.dram_tensor("d", [ROWS_PER_CORE * W], f32, kind="ExternalInput").ap()
    o = nc.dram_tensor("o", [ROWS_PER_CORE * W * 4], f32, kind="ExternalOutput").ap()

    uv = u.rearrange("(t p f) -> t p f", t=N_ITER, p=P)    # [16,128,2048]
    dv = d.rearrange("(t p f) -> t p f", t=N_ITER, p=P)
    ov = o.rearrange("(t p f) -> t p f", t=N_ITER, p=P)    # [16,128,8192]

    with tile.TileContext(nc) as tc, ExitStack() as ctx:
        inp = ctx.enter_context(tc.tile_pool(name="inp", bufs=4))
        outp = ctx.enter_context(tc.tile_pool(name="outp", bufs=3))
        for t in range(N_ITER):
            ut = inp.tile([P, FIN], f32, tag="u")
            dt_ = inp.tile([P, FIN], f32, tag="d")
            nc.sync.dma_start(ut[:], uv[t])
            nc.sync.dma_start(dt_[:], dv[t])

            ot = outp.tile([P, FOUT], f32)
            # free dim of ot: (k, eo, j, g) = (8, 2, 256, 2); out row 2k+eo,
            # element 2j+g.
            o5 = ot[:].rearrange("p (k eo j g) -> p k eo j g", k=K, eo=2, j=W, g=2)
            u3 = ut[:].rearrange("p (k j) -> p k j", k=K)
            d3 = dt_[:].rearrange("p (k j) -> p k j", k=K)
            nc.vector.tensor_copy(o5[:, :, 0, :, 0], d3)   # even row: d,u,d,u..
            nc.scalar.copy(o5[:, :, 0, :, 1], u3)
            nc.vector.tensor_copy(o5[:, :, 1, :, 0], u3)   # odd row: u,d,u,d..
            nc.scalar.copy(o5[:, :, 1, :, 1], d3)

            # Output store on the ACT HWDGE ring so input loads (SP ring)
            # and stores issue in parallel.
            nc.scalar.dma_start(ov[t], ot[:])
    nc.compile()
    return nc


def _get_nc():
    if "nc" not in _cache:
        _cache["nc"] = _build_nc()
    return _cache["nc"]


def run(up_diagonal, down_diagonal, **spmd_kwargs):
    """Run the kernel on 8 cores; returns (out [B,C,2H,2W], BassKernelResults)."""
    u = np.ascontiguousarray(np.asarray(up_diagonal, dtype=np.float32))
    d = np.ascontiguousarray(np.asarray(down_diagonal, dtype=np.float32))
    assert u.shape == (B, C, H, W) and d.shape == (B, C, H, W)
    nc = _get_nc()
    in_maps = [
        {"u": u[k].reshape(-1), "d": d[k].reshape(-1)} for k in range(N_CORES)
    ]
    res = bass_utils.run_bass_kernel_spmd(
        nc, in_maps, core_ids=list(range(N_CORES)), **spmd_kwargs
    )
    out = np.stack([res.results[k]["o"].reshape(C, 2 * H, 2 * W) for k in range(N_CORES)])
    return out, res


def kernel(up_diagonal, down_diagonal):
    out, _ = run(up_diagonal, down_diagonal)
    return out


# revision 9
# speedup vs baseline: 1.1595x; 1.1595x over previous
"""DiagonalUpsample Trainium2 kernel.

Interleave two [B,C,H,W] f32 tensors into [B,C,2H,2W]:
  out[2i,2j]   = d[i,j]    out[2i,2j+1] = u[i,j]
  out[2i+1,2j] = u[i,j]    out[2i+1,2j+1] = d[i,j]

Flattened over rows r = (b,c,i):  out_row[2r] = interleave(d_r, u_r),
out_row[2r+1] = interleave(u_r, d_r).  Sharded batch-parallel across 8
NeuronCores (core k takes batch k).  Per core: 16 iterations; each DMAs
1 MiB of u and d into SBUF ([128 partitions x 2048 f32], partition p
holding 8 consecutive input rows), builds the interleaved output tile
[128 x 8192] with 4 strided copies (DVE + ACT), and stores it with a
single fully-contiguous 4 MiB DMA (partition p's 32 KiB lands exactly at
output rows 16p..16p+15 of the chunk).
"""

import sys

import numpy as np

try:
    import concourse.bass as bass  # noqa: F401
except ImportError:  # grading env may not have the repo on sys.path
    sys.path.insert(0, "/opt/trn_rl_repo")

from contextlib import ExitStack

import concourse.bacc as bacc
import concourse.bass as bass
import concourse.tile as tile
from concourse import mybir
from concourse import bass_utils

B, C, H, W = 8, 64, 256, 256
N_CORES = 8
P = 128                      # SBUF partitions
ROWS_PER_CORE = C * H        # 16384 input rows of W f32 per core
K = 8                        # input rows per partition per iteration
ROWS_PER_ITER = P * K        # 1024
N_ITER = ROWS_PER_CORE // ROWS_PER_ITER  # 16
FIN = K * W                  # 2048 f32 per partition in
FOUT = 4 * FIN               # 8192 f32 per partition out

_cache = {}


def _build_nc():
    nc = bacc.Bacc(
        "TRN2",
        target_bir_lowering=False,
        debug=False,
        enable_asserts=False,
    )
    f32 = mybir.dt.float32
    u = nc.dram_tensor("u", [ROWS_PER_CORE * W], f32, kind="ExternalInput").ap()
    d = nc.dram_tensor("d", [ROWS_PER_CORE * W], f32, kind="ExternalInput").ap()
    o = nc.dram_tensor("o", [ROWS_PER_CORE * W * 4], f32, kind="ExternalOutput").ap()

    uv = u.rearrange("(t p f) -> t p f", t=N_ITER, p=P)    # [16,128,2048]
    dv = d.rearrange("(t p f) -> t p f", t=N_ITER, p=P)
    ov = o.rearrange("(t p f) -> t p f", t=N_ITER, p=P)    # [16,128,8192]

    with tile.TileContext(nc) as tc, ExitStack() as ctx:
        inp = ctx.enter_context(tc.tile_pool(name="inp", bufs=4))
        outp = ctx.enter_context(tc.tile_pool(name="outp", bufs=3))
        for t in range(N_ITER):
            ut = inp.tile([P, FIN], f32, tag="u")
            dt_ = inp.tile([P, FIN], f32, tag="d")
            nc.sync.dma_start(ut[:], uv[t])
            nc.sync.dma_start(dt_[:], dv[t])

            ot = outp.tile([P, FOUT], f32)
            # free dim of ot: (k, eo, j, g) = (8, 2, 256, 2); out row 2k+eo,
            # element 2j+g.
            o5 = ot[:].rearrange("p (k eo j g) -> p k eo j g", k=K, eo=2, j=W, g=2)
            u3 = ut[:].rearrange("p (k j) -> p k j", k=K)
            d3 = dt_[:].rearrange("p (k j) -> p k j", k=K)
            nc.vector.tensor_copy(o5[:, :, 0, :, 0], d3)   # even row: d,u,d,u..
            nc.scalar.copy(o5[:, :, 0, :, 1], u3)
            nc.vector.tensor_copy(o5[:, :, 1, :, 0], u3)   # odd row: u,d,u,d..
            nc.scalar.copy(o5[:, :, 1, :, 1], d3)

            # Output store on the ACT HWDGE ring so input loads (SP ring)
            # and stores issue in parallel.
            nc.scalar.dma_start(ov[t], ot[:])
    nc.compile()
    return nc


def _get_nc():
    if "nc" not in _cache:
        _cache["nc"] = _build_nc()
    return _cache["nc"]


def run(up_diagonal, down_diagonal, **spmd_kwargs):
    """Run the kernel on 8 cores; returns (out [B,C,2H,2W], BassKernelResults)."""
    u = np.ascontiguousarray(np.asarray(up_diagonal, dtype=np.float32))
    d = np.ascontiguousarray(np.asarray(down_diagonal, dtype=np.float32))
    assert u.shape == (B, C, H, W) and d.shape == (B, C, H, W)
    nc = _get_nc()
    in_maps = [
        {"u": u[k].reshape(-1), "d": d[k].reshape(-1)} for k in range(N_CORES)
    ]
    res = None
    for attempt in range(3):
        try:
            res = bass_utils.run_bass_kernel_spmd(
                nc, in_maps, core_ids=list(range(N_CORES)), **spmd_kwargs
            )
            break
        except Exception:
            # The first execution of a freshly compiled NEFF occasionally
            # fails with NRT_EXEC_UNIT_UNRECOVERABLE; a retry succeeds.
            if attempt == 2:
                raise
    assert res is not None
    out = np.stack([res.results[k]["o"].reshape(C, 2 * H, 2 * W) for k in range(N_CORES)])
    return out, res


def kernel(up_diagonal, down_diagonal):
    out, _ = run(up_diagonal, down_diagonal)
    return out
